# revision 3
# baseline (speedup 1.0000x reference)
"""Cross-attention Trainium2 kernel (self-contained).

Reference computation (B=4, N=M=2048, DIM=1024, H=16, Dh=64):
    q = x @ Wq.T ; k = ctx @ Wk.T ; v = ctx @ Wv.T       (per-head split)
    out = softmax(q k^T / sqrt(Dh)) v                     (per b, h)
    final = out @ Wo.T + bo

Sharding over 8 NeuronCores: core c -> (batch b = c//2, head-group g = c%2).
Each core handles 8 heads (512 of the 1024 inner dims) of one batch and
produces a partial (2048, 1024) output-projection contribution; the host sums
the two partials per batch and adds the bias.

Schedule: the ScalarE exp stream (256 tiles x ~1.15us) is the critical
resource; every other engine is paced around it.  A single linearized step
loop (step s = (pr, j, i)) emits per step: scores(s+2) [PE, 2 row-tiled
concurrent matmuls], exp(s) [ACT], attn@V(s) [PE, 2 matmuls incl. the ones
row for the softmax denominator], plus ~2 x 213ns quanta of paced projection
or output-projection work chosen by deadline.  DMAs are ordered first-use-
first in fine chunks so the first exp fires ~17us in.
"""

import numpy as np
import ml_dtypes
from contextlib import ExitStack

import concourse.bass as bass
import concourse.bacc as bacc
import concourse.tile as tile
from concourse import mybir
from concourse import bass_utils

F32 = mybir.dt.float32
BF16 = mybir.dt.bfloat16

B, N, M, DIM = 4, 2048, 2048, 1024
H, DH = 16, 64
NCORES = 8
HG = DIM // 2          # head dims per core (8 heads * 64)
SCALE = DH ** -0.5

NT = N // 512          # q-row tiles of 512 (4)
MT = M // 128          # context-row tiles of 128 (16)
CT = DIM // 128        # contraction tiles for projections (8)
DT = HG // 128         # head-pair tiles per core (4)
NSTEP = DT * NT * MT   # 256 linearized attention steps

_CACHE = {}


def _build_program():
    nc = bacc.Bacc(
        "TRN2",
        target_bir_lowering=False,
        debug=False,
        enable_asserts=False,
        num_devices=NCORES,
    )
    xT = nc.dram_tensor("xT", (DIM, N), BF16, kind="ExternalInput").ap()
    ctxT = nc.dram_tensor("ctxT", (DIM, M), BF16, kind="ExternalInput").ap()
    wqT = nc.dram_tensor("wqT", (DIM, HG), BF16, kind="ExternalInput").ap()
    wkT = nc.dram_tensor("wkT", (DIM, HG), BF16, kind="ExternalInput").ap()
    wvT = nc.dram_tensor("wvT", (DIM, HG), BF16, kind="ExternalInput").ap()
    woT = nc.dram_tensor("woT", (HG, DIM), BF16, kind="ExternalInput").ap()
    out = nc.dram_tensor("out", (N, DIM), F32, kind="ExternalOutput").ap()

    with tile.TileContext(nc) as tc:
        _kernel_body(tc, xT, ctxT, wqT, wkT, wvT, woT, out)
    nc.compile()
    return nc


def _kernel_body(tc, xT, ctxT, wqT, wkT, wvT, woT, out):
    nc = tc.nc
    EXP = mybir.ActivationFunctionType.Exp

    with ExitStack() as ctx:
        sb = ctx.enter_context(tc.tile_pool(name="sb", bufs=1))

        xT_sb = sb.tile([128, CT, N], BF16, tag="xT")
        ctxT_sb = sb.tile([128, CT, M], BF16, tag="ctxT")
        wq_sb = sb.tile([128, CT, HG], BF16, tag="wq")
        wk_sb = sb.tile([128, CT, HG], BF16, tag="wk")
        wv_sb = sb.tile([128, CT, HG], BF16, tag="wv")
        wo_sb = sb.tile([128, DT, DIM], BF16, tag="wo")
        qT_sb = sb.tile([128, DT, N], BF16, tag="qT")
        kT_sb = sb.tile([128, DT, M], BF16, tag="kT")
        v_sb = sb.tile([128, MT, 8 * 65], BF16, tag="v")
        on_sb = sb.tile([128, DT, N], BF16, tag="on")

        # ---- DMA emission: first-use-first, 512-col chunks ----
        def load_w(dst, src):
            for c in range(CT):
                nc.sync.dma_start(out=dst[:, c, :], in_=src[c * 128:(c + 1) * 128, :])

        def load_cols(dst, src, lo, hi):
            for c in range(CT):
                nc.sync.dma_start(out=dst[:, c, lo:hi],
                                  in_=src[c * 128:(c + 1) * 128, lo:hi])

        load_w(wq_sb, wqT)
        load_cols(xT_sb, xT, 0, 512)          # q(0,0)
        load_w(wk_sb, wkT)
        load_cols(ctxT_sb, ctxT, 0, 512)      # k(0,0), v(0, 0..3)
        load_w(wv_sb, wvT)
        load_cols(ctxT_sb, ctxT, 512, 1024)   # k(0,1), v(0, 4..7)
        load_cols(ctxT_sb, ctxT, 1024, 1536)  # v(0, 8..11)
        load_cols(xT_sb, xT, 512, 1024)       # q(0,1)
        load_cols(ctxT_sb, ctxT, 1536, 2048)  # k(0,2..3), v(0, 12..15)
        load_cols(xT_sb, xT, 1024, 2048)      # q(0,2..3)
        for t in range(DT):
            nc.sync.dma_start(out=wo_sb[:, t, :], in_=woT[t * 128:(t + 1) * 128, :])

        v_r = v_sb.rearrange("p m (h x) -> p m h x", x=65)
        for h in range(8):
            nc.vector.memset(v_r[:, :, h, 64:65], 1.0)

        # ---- pools ----
        psp = ctx.enter_context(tc.tile_pool(name="psp", bufs=2, space="PSUM"))
        pss = ctx.enter_context(tc.tile_pool(name="pss", bufs=2, space="PSUM"))
        pso = ctx.enter_context(tc.tile_pool(name="pso", bufs=2, space="PSUM"))
        sba = ctx.enter_context(tc.tile_pool(name="sba", bufs=4))
        sbn = ctx.enter_context(tc.tile_pool(name="sbn", bufs=4))
        sbo = ctx.enter_context(tc.tile_pool(name="sbo", bufs=3))

        # ---- paced work generators (yield ~213ns PE quanta) ----
        def q_group(pr, jn):
            ps = psp.tile([128, 512], F32, tag="proj", name="qg")
            for c in range(CT):
                nc.tensor.matmul(
                    ps,
                    wq_sb[:, c, pr * 128:(pr + 1) * 128],
                    xT_sb[:, c, jn * 512:(jn + 1) * 512],
                    start=(c == 0), stop=(c == CT - 1),
                )
                yield
            nc.vector.tensor_copy(qT_sb[:, pr, jn * 512:(jn + 1) * 512], ps)

        def k_group(pr, jm):
            ps = psp.tile([128, 512], F32, tag="proj", name="kg")
            for c in range(CT):
                nc.tensor.matmul(
                    ps,
                    wk_sb[:, c, pr * 128:(pr + 1) * 128],
                    ctxT_sb[:, c, jm * 512:(jm + 1) * 512],
                    start=(c == 0), stop=(c == CT - 1),
                )
                yield
            nc.vector.tensor_copy(kT_sb[:, pr, jm * 512:(jm + 1) * 512], ps)

        def v_group(cp, i):
            # one couple = two head pairs = 4 heads (256 projection dims)
            ps = psp.tile([128, 256], F32, tag="proj", name="vg")
            for c in range(CT):
                nc.tensor.matmul(
                    ps,
                    ctxT_sb[:, c, i * 128:(i + 1) * 128],
                    wv_sb[:, c, cp * 256:(cp + 1) * 256],
                    start=(c == 0), stop=(c == CT - 1),
                )
                if c % 2 == 1:
                    yield
            nc.vector.tensor_copy(
                v_r[:, i, 4 * cp:4 * cp + 4, 0:64],
                ps.rearrange("p (h d) -> p h d", h=4),
            )

        def final_group(n128, e):
            ps = psp.tile([128, 512], F32, tag="proj", name="fg")
            for t in range(DT):
                nc.tensor.matmul(
                    ps,
                    on_sb[:, t, n128 * 128:(n128 + 1) * 128],
                    wo_sb[:, t, e * 512:(e + 1) * 512],
                    start=(t == 0), stop=(t == DT - 1),
                )
                yield
            of = sbo.tile([128, 512], F32, tag="of", name="of")
            nc.vector.tensor_copy(of, ps)
            nc.sync.dma_start(
                out=out[n128 * 128:(n128 + 1) * 128, e * 512:(e + 1) * 512],
                in_=of,
            )

        # ---- deadline-ordered pacer ----
        class Pacer:
            # At most one group is ever mid-emission (self.cur); a suspended
            # group is always finished before any other group starts, so the
            # 2-buffer proj-PSUM ring never wraps onto a live accumulation.
            def __init__(self):
                self.items = []   # list of [deadline, avail, gen], queue order
                self.cur = None
                self._cur_dl = 10 ** 9

            def add(self, dl, avail, gen):
                self.items.append([dl, avail, gen])

            def sort(self):
                self.items.sort(key=lambda it: it[0])

            def run_due(self, s):
                # fully drain anything whose deadline has arrived
                due = any(it[0] <= s for it in self.items) or (
                    self.cur is not None and self._cur_dl <= s)
                if due and self.cur is not None:
                    for _ in self.cur:
                        pass
                    self.cur = None
                while True:
                    hit = None
                    for idx, it in enumerate(self.items):
                        if it[0] <= s:
                            hit = idx
                            break
                    if hit is None:
                        break
                    for _ in self.items.pop(hit)[2]:
                        pass

            def step(self, s, budget):
                for _ in range(budget):
                    while True:
                        if self.cur is None:
                            nxt = None
                            for idx, it in enumerate(self.items):
                                if it[1] <= s:
                                    nxt = self.items.pop(idx)
                                    break
                            if nxt is None:
                                return
                            self._cur_dl = nxt[0]
                            self.cur = nxt[2]
                        try:
                            next(self.cur)
                            break
                        except StopIteration:
                            self.cur = None

            def drain(self):
                if self.cur is not None:
                    for _ in self.cur:
                        pass
                    self.cur = None
                for it in self.items:
                    for _ in it[2]:
                        pass
                self.items = []

        pacer = Pacer()

        def sp(pr, j, i):
            return pr * 64 + j * 16 + i

        # projections not covered by the upfront batch, with deadlines
        for jn in range(1, NT):
            pacer.add(sp(0, jn, 0) - 2, -1, q_group(0, jn))
        for jm in range(2, 4):
            pacer.add(sp(0, 0, 4 * jm - 2), -1, k_group(0, jm))
        for i in range(12, MT):
            pacer.add(sp(0, 0, i), -1, v_group(0, i))
        for pr in range(1, DT):
            for jn in range(NT):
                pacer.add(sp(pr, jn, 0) - 2, -1, q_group(pr, jn))
            for jm in range(4):
                pacer.add(max(0, sp(pr, 0, 4 * jm - 2)), -1, k_group(pr, jm))
        for i in range(MT):
            pacer.add(sp(2, 0, i), -1, v_group(1, i))
        pacer.sort()
        # output projection for row block jj becomes available after
        # normalize(3, jj); deadline far in the future (opportunistic).
        for jj in range(NT):
            av = sp(3, jj, 15) + 1
            for n128 in range(jj * 4, jj * 4 + 4):
                for e in range(2):
                    pacer.add(10 ** 6, av, final_group(n128, e))

        # ---- upfront batch (hidden under the initial DMA wait) ----
        for g in ([q_group(0, 0), k_group(0, 0), k_group(0, 1)]
                  + [v_group(0, i) for i in range(12)]):
            for _ in g:
                pass

        # ---- attention step machinery ----
        def scores(pr, j, i):
            s = pss.tile([128, 1024], F32, tag="sc", name="sc")
            for half in range(2):
                lo, hi = half * 64, half * 64 + 64
                nc.tensor.matmul(
                    s[:, half * 512:(half + 1) * 512],
                    kT_sb[lo:hi, pr, i * 128:(i + 1) * 128],
                    qT_sb[lo:hi, pr, j * 512:(j + 1) * 512],
                    start=True, stop=True,
                )
            return s

        def exp_av(oo, pr, i, s):
            a = sba.tile([128, 1024], BF16, tag="attn", name="attn")
            nc.scalar.activation(a, s, EXP, scale=SCALE)
            for half in range(2):
                nc.tensor.matmul(
                    oo[half],
                    v_r[:, i, 2 * pr + half, :],
                    a[:, half * 512:(half + 1) * 512],
                    start=(i == 0), stop=(i == MT - 1),
                )

        def normalize(oo, pr, j):
            # rows 0..63 of oacc are sum(attn*v); row 64 is sum(attn).
            bcs = []
            for half in range(2):
                den = sbn.tile([1, 512], F32, tag="den", name="den")
                nc.vector.tensor_copy(den, oo[half][64:65, :])
                rec32 = sbn.tile([1, 512], F32, tag="rec32", name="rec32")
                nc.vector.reciprocal_approx_fast(out=rec32, in_=den)
                bc = sbn.tile([64, 512], F32, tag="bc", name="bc")
                nc.gpsimd.partition_broadcast(bc, rec32)
                bcs.append(bc)
            for half in range(2):
                nc.vector.tensor_mul(
                    on_sb[half * 64:half * 64 + 64, pr, j * 512:(j + 1) * 512],
                    oo[half][0:64, :], bcs[half],
                )

        # ---- linearized main loop ----
        fifo = [scores(0, 0, 0), scores(0, 0, 1)]
        oo = None
        for s in range(NSTEP):
            pr, j, i = s // 64, (s // 16) % 4, s % 16
            pacer.run_due(s)
            if i == 0:
                oo = [pso.tile([65, 512], F32, tag="oacc", name=f"oacc{h}")
                      for h in range(2)]
            if s + 2 < NSTEP:
                s2 = s + 2
                fifo.append(scores(s2 // 64, (s2 // 16) % 4, s2 % 16))
            exp_av(oo, pr, i, fifo.pop(0))
            pacer.step(s, 2)
            if i == 15:
                normalize(oo, pr, j)
        pacer.drain()


def kernel(x, context, Wq, Wk, Wv, Wo, bo):
    x = np.asarray(x, dtype=np.float32)
    context = np.asarray(context, dtype=np.float32)
    Wq = np.asarray(Wq, dtype=np.float32)
    Wk = np.asarray(Wk, dtype=np.float32)
    Wv = np.asarray(Wv, dtype=np.float32)
    Wo = np.asarray(Wo, dtype=np.float32)
    bo = np.asarray(bo, dtype=np.float32)

    if "nc" not in _CACHE:
        _CACHE["nc"] = _build_program()
    nc = _CACHE["nc"]

    in_maps = _make_in_maps(x, context, Wq, Wk, Wv, Wo)
    res = bass_utils.run_bass_kernel_spmd(nc, in_maps, core_ids=list(range(NCORES)))

    final = np.empty((B, N, DIM), dtype=np.float32)
    for b in range(B):
        final[b] = res.results[2 * b]["out"] + res.results[2 * b + 1]["out"] + bo
    return final


def _make_in_maps(x, context, Wq, Wk, Wv, Wo):
    bf = ml_dtypes.bfloat16
    xT = [np.ascontiguousarray(x[b].T).astype(bf) for b in range(B)]
    ctxT = [np.ascontiguousarray(context[b].T).astype(bf) for b in range(B)]
    wT = {}
    for g in range(2):
        sl = slice(g * HG, (g + 1) * HG)
        wT[g] = {
            "wqT": np.ascontiguousarray(Wq[sl, :].T).astype(bf),
            "wkT": np.ascontiguousarray(Wk[sl, :].T).astype(bf),
            "wvT": np.ascontiguousarray(Wv[sl, :].T).astype(bf),
            "woT": np.ascontiguousarray(Wo[:, sl].T).astype(bf),
        }
    in_maps = []
    for c in range(NCORES):
        b, g = c // 2, c % 2
        m = {"xT": xT[b], "ctxT": ctxT[b]}
        m.update(wT[g])
        in_maps.append(m)
    return in_maps


def timed_run(inp, trace_dir=None):
    """Run with NTFF tracing; returns HW exec time in ns (or None)."""
    if "nc" not in _CACHE:
        _CACHE["nc"] = _build_program()
    nc = _CACHE["nc"]
    in_maps = _make_in_maps(
        np.asarray(inp["x"], np.float32), np.asarray(inp["context"], np.float32),
        np.asarray(inp["Wq"], np.float32), np.asarray(inp["Wk"], np.float32),
        np.asarray(inp["Wv"], np.float32), np.asarray(inp["Wo"], np.float32))
    res = bass_utils.run_bass_kernel_spmd(
        nc, in_maps, core_ids=list(range(NCORES)), trace=True, tmpdir=trace_dir)
    return res.exec_time_ns


# revision 4
# speedup vs baseline: 1.2091x; 1.2091x over previous
"""Cross-attention Trainium2 kernel (self-contained).

Reference computation (B=4, N=M=2048, DIM=1024, H=16, Dh=64):
    q = x @ Wq.T ; k = ctx @ Wk.T ; v = ctx @ Wv.T       (per-head split)
    out = softmax(q k^T / sqrt(Dh)) v                     (per b, h)
    final = out @ Wo.T + bo

Sharding over 8 NeuronCores: core c -> (batch b = c//2, head-group g = c%2).
Each core handles 8 heads (512 of the 1024 inner dims) of one batch and
produces a partial (2048, 1024) output-projection contribution; the host sums
the two partials per batch and adds the bias.

Schedule: the ScalarE exp stream (256 tiles x ~1.15us) is the critical
resource.  A linearized step loop (s -> (pr, j, i)) emits per step:
scores(s+2) [PE, 2 row-tiled concurrent matmuls], exp(s) [ACT], attn@V(s-4)
[PE, lagged 4 steps so early-DMA gaps never block the exp chain], plus ~2.25
x 213ns quanta of deadline-paced projection / output-projection work.
Inputs are staged c-major in DRAM so each tensor loads with 1-4 large DMAs
(~1MB each); load order is first-use-first so the first exp fires ~19us in.
"""

import numpy as np
import ml_dtypes
from contextlib import ExitStack

import concourse.bass as bass
import concourse.bacc as bacc
import concourse.tile as tile
from concourse import mybir
from concourse import bass_utils

F32 = mybir.dt.float32
BF16 = mybir.dt.bfloat16

B, N, M, DIM = 4, 2048, 2048, 1024
H, DH = 16, 64
NCORES = 8
HG = DIM // 2          # head dims per core (8 heads * 64)
SCALE = DH ** -0.5

NT = N // 512          # q-row tiles of 512 (4)
MT = M // 128          # context-row tiles of 128 (16)
CT = DIM // 128        # contraction tiles for projections (8)
DT = HG // 128         # head-pair tiles per core (4)
NSTEP = DT * NT * MT   # 256 linearized attention steps
LAG = 4                # attn@V emission lag (steps)

_CACHE = {}


def _build_program():
    nc = bacc.Bacc(
        "TRN2",
        target_bir_lowering=False,
        debug=False,
        enable_asserts=False,
        num_devices=NCORES,
    )
    # c-major staging so whole tensors load in 1-4 large DMA instructions
    xT = nc.dram_tensor("xT", (CT, 128, N), BF16, kind="ExternalInput").ap()
    ctxT = nc.dram_tensor("ctxT", (CT, 128, M), BF16, kind="ExternalInput").ap()
    wqT = nc.dram_tensor("wqT", (CT, 128, HG), BF16, kind="ExternalInput").ap()
    wkT = nc.dram_tensor("wkT", (CT, 128, HG), BF16, kind="ExternalInput").ap()
    wvT = nc.dram_tensor("wvT", (CT, 128, HG), BF16, kind="ExternalInput").ap()
    woT = nc.dram_tensor("woT", (DT, 128, DIM), BF16, kind="ExternalInput").ap()
    out = nc.dram_tensor("out", (N, DIM), F32, kind="ExternalOutput").ap()

    with tile.TileContext(nc) as tc:
        _kernel_body(tc, xT, ctxT, wqT, wkT, wvT, woT, out)
    nc.compile()
    return nc


def _kernel_body(tc, xT, ctxT, wqT, wkT, wvT, woT, out):
    nc = tc.nc
    EXP = mybir.ActivationFunctionType.Exp

    with ExitStack() as ctx:
        sb = ctx.enter_context(tc.tile_pool(name="sb", bufs=1))

        xT_sb = sb.tile([128, CT, N], BF16, tag="xT")
        ctxT_sb = sb.tile([128, CT, M], BF16, tag="ctxT")
        wq_sb = sb.tile([128, CT, HG], BF16, tag="wq")
        wk_sb = sb.tile([128, CT, HG], BF16, tag="wk")
        wv_sb = sb.tile([128, CT, HG], BF16, tag="wv")
        wo_sb = sb.tile([128, DT, DIM], BF16, tag="wo")
        qT_sb = sb.tile([128, DT, N], BF16, tag="qT")
        kT_sb = sb.tile([128, DT, M], BF16, tag="kT")
        v_sb = sb.tile([128, MT, 8 * 65], BF16, tag="v")
        on_sb = sb.tile([128, DT, N], BF16, tag="on")

        sbn = ctx.enter_context(tc.tile_pool(name="sbn", bufs=4))

        # warm the ACT exp table while DMAs stream (saves ~2.7us later)
        dumin = sbn.tile([1, 8], F32, tag="dumin", name="dumin")
        nc.vector.memset(dumin, 0.0)
        dumout = sbn.tile([1, 8], F32, tag="dumout", name="dumout")
        nc.scalar.activation(dumout, dumin, EXP, scale=1.0)

        # ---- DMA emission: first-use-first, ~1MB per instruction ----
        def load_all(dst, src):
            nc.sync.dma_start(out=dst.rearrange("p c f -> c p f"), in_=src)

        def load_cols(dst, src, lo, hi):
            nc.sync.dma_start(out=dst[:, :, lo:hi].rearrange("p c f -> c p f"),
                              in_=src[:, :, lo:hi])

        load_all(wq_sb, wqT)
        load_cols(xT_sb, xT, 0, 512)          # q(0,0)
        load_all(wk_sb, wkT)
        load_cols(ctxT_sb, ctxT, 0, 512)      # k(0,0)
        load_cols(ctxT_sb, ctxT, 512, 1024)   # k(0,1)
        load_all(wv_sb, wvT)                  # v(0, 0..7)
        load_cols(ctxT_sb, ctxT, 1024, 1536)  # k(0,2), v(0, 8..11)
        load_cols(ctxT_sb, ctxT, 1536, 2048)  # k(0,3), v(0, 12..15)
        load_cols(xT_sb, xT, 512, 1024)       # q(0,1)
        load_cols(xT_sb, xT, 1024, 2048)      # q(0,2..3)
        load_all(wo_sb, woT)

        v_r = v_sb.rearrange("p m (h x) -> p m h x", x=65)
        for h in range(8):
            nc.vector.memset(v_r[:, :, h, 64:65], 1.0)

        # ---- pools ----
        psp = ctx.enter_context(tc.tile_pool(name="psp", bufs=2, space="PSUM"))
        pss = ctx.enter_context(tc.tile_pool(name="pss", bufs=2, space="PSUM"))
        pso = ctx.enter_context(tc.tile_pool(name="pso", bufs=2, space="PSUM"))
        sba = ctx.enter_context(tc.tile_pool(name="sba", bufs=8))
        sbo = ctx.enter_context(tc.tile_pool(name="sbo", bufs=3))

        # ---- paced work generators (yield ~213ns PE quanta) ----
        def q_group(pr, jn):
            ps = psp.tile([128, 512], F32, tag="proj", name="qg")
            for c in range(CT):
                nc.tensor.matmul(
                    ps,
                    wq_sb[:, c, pr * 128:(pr + 1) * 128],
                    xT_sb[:, c, jn * 512:(jn + 1) * 512],
                    start=(c == 0), stop=(c == CT - 1),
                )
                yield
            nc.vector.tensor_copy(qT_sb[:, pr, jn * 512:(jn + 1) * 512], ps)

        def k_group(pr, jm):
            ps = psp.tile([128, 512], F32, tag="proj", name="kg")
            for c in range(CT):
                nc.tensor.matmul(
                    ps,
                    wk_sb[:, c, pr * 128:(pr + 1) * 128],
                    ctxT_sb[:, c, jm * 512:(jm + 1) * 512],
                    start=(c == 0), stop=(c == CT - 1),
                )
                yield
            nc.vector.tensor_copy(kT_sb[:, pr, jm * 512:(jm + 1) * 512], ps)

        def v_group(cp, i):
            # one couple = two head pairs = 4 heads (256 projection dims)
            ps = psp.tile([128, 256], F32, tag="proj", name="vg")
            for c in range(CT):
                nc.tensor.matmul(
                    ps,
                    ctxT_sb[:, c, i * 128:(i + 1) * 128],
                    wv_sb[:, c, cp * 256:(cp + 1) * 256],
                    start=(c == 0), stop=(c == CT - 1),
                )
                if c % 2 == 1:
                    yield
            nc.vector.tensor_copy(
                v_r[:, i, 4 * cp:4 * cp + 4, 0:64],
                ps.rearrange("p (h d) -> p h d", h=4),
            )

        def final_group(n128, e):
            ps = psp.tile([128, 512], F32, tag="proj", name="fg")
            for t in range(DT):
                nc.tensor.matmul(
                    ps,
                    on_sb[:, t, n128 * 128:(n128 + 1) * 128],
                    wo_sb[:, t, e * 512:(e + 1) * 512],
                    start=(t == 0), stop=(t == DT - 1),
                )
                yield
            of = sbo.tile([128, 512], F32, tag="of", name="of")
            nc.vector.tensor_copy(of, ps)
            nc.sync.dma_start(
                out=out[n128 * 128:(n128 + 1) * 128, e * 512:(e + 1) * 512],
                in_=of,
            )

        # ---- deadline-ordered pacer ----
        class Pacer:
            # At most one group is ever mid-emission (self.cur); a suspended
            # group is always finished before any other group starts, so the
            # 2-buffer proj-PSUM ring never wraps onto a live accumulation.
            def __init__(self):
                self.items = []   # list of [deadline, avail, gen]
                self.cur = None
                self._cur_dl = 10 ** 9

            def add(self, dl, avail, gen):
                self.items.append([dl, avail, gen])

            def sort(self):
                self.items.sort(key=lambda it: it[0])

            def run_due(self, s):
                due = any(it[0] <= s for it in self.items) or (
                    self.cur is not None and self._cur_dl <= s)
                if due and self.cur is not None:
                    for _ in self.cur:
                        pass
                    self.cur = None
                while True:
                    hit = None
                    for idx, it in enumerate(self.items):
                        if it[0] <= s:
                            hit = idx
                            break
                    if hit is None:
                        break
                    for _ in self.items.pop(hit)[2]:
                        pass

            def step(self, s, budget):
                for _ in range(budget):
                    while True:
                        if self.cur is None:
                            nxt = None
                            for idx, it in enumerate(self.items):
                                if it[1] <= s:
                                    nxt = self.items.pop(idx)
                                    break
                            if nxt is None:
                                return
                            self._cur_dl = nxt[0]
                            self.cur = nxt[2]
                        try:
                            next(self.cur)
                            break
                        except StopIteration:
                            self.cur = None

            def drain(self):
                if self.cur is not None:
                    for _ in self.cur:
                        pass
                    self.cur = None
                for it in self.items:
                    for _ in it[2]:
                        pass
                self.items = []

        pacer = Pacer()

        def sp(pr, j, i):
            return pr * 64 + j * 16 + i

        # pair-0 projections beyond the upfront batch (avail = conservative
        # DMA-arrival step so opportunistic runs never stall the PE queue)
        pacer.add(2, 0, k_group(0, 1))
        pacer.add(6, 5, k_group(0, 2))
        pacer.add(10, 7, k_group(0, 3))
        for i in range(MT):
            av = 3 if i < 8 else (5 if i < 12 else 7)
            pacer.add(i + LAG, av, v_group(0, i))
        pacer.add(14, 10, q_group(0, 1))
        pacer.add(30, 14, q_group(0, 2))
        pacer.add(46, 14, q_group(0, 3))
        # later pairs
        for pr in range(1, DT):
            for jn in range(NT):
                pacer.add(sp(pr, jn, 0) - 2, 14, q_group(pr, jn))
            for jm in range(4):
                pacer.add(sp(pr, 0, 4 * jm) - 2, 8, k_group(pr, jm))
        for i in range(MT):
            pacer.add(sp(2, 0, i) + LAG, 8, v_group(1, i))
        pacer.sort()
        # output projection for row block jj: available once normalize(3,jj)
        # has been emitted (step 16*jj + 211 + 1); deadline opportunistic.
        for jj in range(NT):
            av = 192 + jj * 16 + 15 + LAG + 1
            for n128 in range(jj * 4, jj * 4 + 4):
                for e in range(2):
                    pacer.add(10 ** 6, av, final_group(n128, e))

        # ---- upfront batch (hidden under the initial DMA wait) ----
        for g in [q_group(0, 0), k_group(0, 0)]:
            for _ in g:
                pass

        # ---- attention step machinery ----
        def scores(pr, j, i):
            s = pss.tile([128, 1024], F32, tag="sc", name="sc")
            for half in range(2):
                lo, hi = half * 64, half * 64 + 64
                nc.tensor.matmul(
                    s[:, half * 512:(half + 1) * 512],
                    kT_sb[lo:hi, pr, i * 128:(i + 1) * 128],
                    qT_sb[lo:hi, pr, j * 512:(j + 1) * 512],
                    start=True, stop=True,
                )
            return s

        def do_exp(s_tile):
            a = sba.tile([128, 1024], BF16, tag="attn", name="attn")
            nc.scalar.activation(a, s_tile, EXP, scale=SCALE)
            return a

        def do_av(oo, pr, i, a):
            for half in range(2):
                nc.tensor.matmul(
                    oo[half],
                    v_r[:, i, 2 * pr + half, :],
                    a[:, half * 512:(half + 1) * 512],
                    start=(i == 0), stop=(i == MT - 1),
                )

        def normalize(oo, pr, j):
            # rows 0..63 of oacc are sum(attn*v); row 64 is sum(attn).
            bcs = []
            for half in range(2):
                den = sbn.tile([1, 512], F32, tag="den", name="den")
                nc.vector.tensor_copy(den, oo[half][64:65, :])
                rec32 = sbn.tile([1, 512], F32, tag="rec32", name="rec32")
                nc.vector.reciprocal_approx_fast(out=rec32, in_=den)
                bc = sbn.tile([64, 512], F32, tag="bc", name="bc")
                nc.gpsimd.partition_broadcast(bc, rec32)
                bcs.append(bc)
            for half in range(2):
                nc.vector.tensor_mul(
                    on_sb[half * 64:half * 64 + 64, pr, j * 512:(j + 1) * 512],
                    oo[half][0:64, :], bcs[half],
                )

        # ---- linearized main loop; AV lags the exp stream by LAG steps ----
        fifo = [scores(0, 0, 0), scores(0, 0, 1)]
        afifo = []
        oo = None

        def av_step(sa):
            nonlocal oo
            pr, j, i = sa // 64, (sa // 16) % 4, sa % 16
            if i == 0:
                oo = [pso.tile([65, 512], F32, tag="oacc", name=f"oacc{h}")
                      for h in range(2)]
            do_av(oo, pr, i, afifo.pop(0))
            if i == 15:
                normalize(oo, pr, j)

        for s in range(NSTEP):
            pacer.run_due(s)
            if s + 2 < NSTEP:
                s2 = s + 2
                fifo.append(scores(s2 // 64, (s2 // 16) % 4, s2 % 16))
            afifo.append(do_exp(fifo.pop(0)))
            if s >= LAG:
                av_step(s - LAG)
            pacer.step(s, 3 if s % 4 == 3 else 2)
        for sa in range(NSTEP - LAG, NSTEP):
            av_step(sa)
        pacer.drain()


def kernel(x, context, Wq, Wk, Wv, Wo, bo):
    x = np.asarray(x, dtype=np.float32)
    context = np.asarray(context, dtype=np.float32)
    Wq = np.asarray(Wq, dtype=np.float32)
    Wk = np.asarray(Wk, dtype=np.float32)
    Wv = np.asarray(Wv, dtype=np.float32)
    Wo = np.asarray(Wo, dtype=np.float32)
    bo = np.asarray(bo, dtype=np.float32)

    if "nc" not in _CACHE:
        _CACHE["nc"] = _build_program()
    nc = _CACHE["nc"]

    in_maps = _make_in_maps(x, context, Wq, Wk, Wv, Wo)
    res = bass_utils.run_bass_kernel_spmd(nc, in_maps, core_ids=list(range(NCORES)))

    final = np.empty((B, N, DIM), dtype=np.float32)
    for b in range(B):
        final[b] = res.results[2 * b]["out"] + res.results[2 * b + 1]["out"] + bo
    return final


def _ctile(a):
    # (rows, cols) -> (rows//128, 128, cols) c-major staging
    return np.ascontiguousarray(a.reshape(-1, 128, a.shape[1]))


def _make_in_maps(x, context, Wq, Wk, Wv, Wo):
    bf = ml_dtypes.bfloat16
    xT = [_ctile(np.ascontiguousarray(x[b].T).astype(bf)) for b in range(B)]
    ctxT = [_ctile(np.ascontiguousarray(context[b].T).astype(bf)) for b in range(B)]
    wT = {}
    for g in range(2):
        sl = slice(g * HG, (g + 1) * HG)
        wT[g] = {
            "wqT": _ctile(np.ascontiguousarray(Wq[sl, :].T).astype(bf)),
            "wkT": _ctile(np.ascontiguousarray(Wk[sl, :].T).astype(bf)),
            "wvT": _ctile(np.ascontiguousarray(Wv[sl, :].T).astype(bf)),
            "woT": _ctile(np.ascontiguousarray(Wo[:, sl].T).astype(bf)),
        }
    in_maps = []
    for c in range(NCORES):
        b, g = c // 2, c % 2
        m = {"xT": xT[b], "ctxT": ctxT[b]}
        m.update(wT[g])
        in_maps.append(m)
    return in_maps


def timed_run(inp, trace_dir=None):
    """Run with NTFF tracing; returns HW exec time in ns (or None)."""
    if "nc" not in _CACHE:
        _CACHE["nc"] = _build_program()
    nc = _CACHE["nc"]
    in_maps = _make_in_maps(
        np.asarray(inp["x"], np.float32), np.asarray(inp["context"], np.float32),
        np.asarray(inp["Wq"], np.float32), np.asarray(inp["Wk"], np.float32),
        np.asarray(inp["Wv"], np.float32), np.asarray(inp["Wo"], np.float32))
    res = bass_utils.run_bass_kernel_spmd(
        nc, in_maps, core_ids=list(range(NCORES)), trace=True, tmpdir=trace_dir)
    return res.exec_time_ns


# revision 8
# speedup vs baseline: 1.2248x; 1.0129x over previous
"""Cross-attention Trainium2 kernel (self-contained).

Reference computation (B=4, N=M=2048, DIM=1024, H=16, Dh=64):
    q = x @ Wq.T ; k = ctx @ Wk.T ; v = ctx @ Wv.T       (per-head split)
    out = softmax(q k^T / sqrt(Dh)) v                     (per b, h)
    final = out @ Wo.T + bo

Sharding over 8 NeuronCores: core c -> (batch b = c//2, head-group g = c%2).
Each core handles 8 heads (512 of the 1024 inner dims) of one batch and
produces a partial (2048, 1024) output-projection contribution; the host sums
the two partials per batch and adds the bias.

Schedule: the ScalarE exp stream (256 tiles x ~1.15us) is the critical
resource.  A linearized step loop (s -> (pr, j, i)) emits per step:
scores(s+2) [PE, 2 row-tiled concurrent matmuls], exp(s) [ACT], attn@V(s-4)
[PE, lagged 4 steps so early-DMA gaps never block the exp chain], plus ~2.25
x 213ns quanta of deadline-paced projection / output-projection work.
Inputs are staged c-major in DRAM so each tensor loads with 1-4 large DMAs
(~1MB each); load order is first-use-first so the first exp fires ~19us in.
"""

import numpy as np
import ml_dtypes
from contextlib import ExitStack

import concourse.bass as bass
import concourse.bacc as bacc
import concourse.tile as tile
from concourse import mybir
from concourse import bass_utils

F32 = mybir.dt.float32
BF16 = mybir.dt.bfloat16

B, N, M, DIM = 4, 2048, 2048, 1024
H, DH = 16, 64
NCORES = 8
HG = DIM // 2          # head dims per core (8 heads * 64)
SCALE = DH ** -0.5

NT = N // 512          # q-row tiles of 512 (4)
MT = M // 128          # context-row tiles of 128 (16)
CT = DIM // 128        # contraction tiles for projections (8)
DT = HG // 128         # head-pair tiles per core (4)
NSTEP = DT * NT * MT   # 256 linearized attention steps
LAG = 4                # attn@V emission lag (steps)

_CACHE = {}


def _build_program():
    nc = bacc.Bacc(
        "TRN2",
        target_bir_lowering=False,
        debug=False,
        enable_asserts=False,
        num_devices=NCORES,
    )
    # inputs staged host-side as SBUF images (partition-major), so each
    # tensor / column-chunk loads with one natural-order DMA instruction
    # with 8KB-per-partition contiguous lines (~435 GB/s).
    xT = nc.dram_tensor("xT", (4, 128, CT, 512), BF16, kind="ExternalInput").ap()
    ctxT = nc.dram_tensor("ctxT", (4, 128, CT, 512), BF16, kind="ExternalInput").ap()
    wqT = nc.dram_tensor("wqT", (128, CT, HG), BF16, kind="ExternalInput").ap()
    wkT = nc.dram_tensor("wkT", (128, CT, HG), BF16, kind="ExternalInput").ap()
    wvT = nc.dram_tensor("wvT", (128, CT, HG), BF16, kind="ExternalInput").ap()
    woT = nc.dram_tensor("woT", (128, DT, DIM), BF16, kind="ExternalInput").ap()
    out = nc.dram_tensor("out", (N, DIM), F32, kind="ExternalOutput").ap()

    with tile.TileContext(nc) as tc:
        _kernel_body(tc, xT, ctxT, wqT, wkT, wvT, woT, out)
    nc.compile()
    return nc


def _kernel_body(tc, xT, ctxT, wqT, wkT, wvT, woT, out):
    nc = tc.nc
    EXP = mybir.ActivationFunctionType.Exp

    with ExitStack() as ctx:
        sb = ctx.enter_context(tc.tile_pool(name="sb", bufs=1))

        xT_sb = sb.tile([128, CT, N], BF16, tag="xT")
        ctxT_sb = sb.tile([128, CT, M], BF16, tag="ctxT")
        wq_sb = sb.tile([128, CT, HG], BF16, tag="wq")
        wk_sb = sb.tile([128, CT, HG], BF16, tag="wk")
        wv_sb = sb.tile([128, CT, HG], BF16, tag="wv")
        wo_sb = sb.tile([128, DT, DIM], BF16, tag="wo")
        qT_sb = sb.tile([128, DT, N], BF16, tag="qT")
        kT_sb = sb.tile([128, DT, M], BF16, tag="kT")
        v_sb = sb.tile([128, MT, 8 * 65], BF16, tag="v")
        on_sb = sb.tile([128, DT, N], BF16, tag="on")

        sbn = ctx.enter_context(tc.tile_pool(name="sbn", bufs=4))

        # warm the ACT exp table while DMAs stream (saves ~2.7us later)
        dumin = sbn.tile([1, 8], F32, tag="dumin", name="dumin")
        nc.vector.memset(dumin, 0.0)
        dumout = sbn.tile([1, 8], F32, tag="dumout", name="dumout")
        nc.scalar.activation(dumout, dumin, EXP, scale=1.0)

        # ---- DMA emission: first-use-first, ~1MB per instruction ----
        def load_chunk(dst, src, ch):
            nc.sync.dma_start(out=dst[:, :, ch * 512:(ch + 1) * 512], in_=src[ch])

        nc.sync.dma_start(out=wq_sb, in_=wqT)
        load_chunk(xT_sb, xT, 0)              # q(0,0)
        nc.sync.dma_start(out=wk_sb, in_=wkT)
        load_chunk(ctxT_sb, ctxT, 0)          # k(0,0)
        load_chunk(ctxT_sb, ctxT, 1)          # k(0,1)
        nc.sync.dma_start(out=wv_sb, in_=wvT)  # v(0, 0..7)
        load_chunk(ctxT_sb, ctxT, 2)          # k(0,2), v(0, 8..11)
        load_chunk(ctxT_sb, ctxT, 3)          # k(0,3), v(0, 12..15)
        load_chunk(xT_sb, xT, 1)              # q(0,1)
        load_chunk(xT_sb, xT, 2)              # q(0,2)
        load_chunk(xT_sb, xT, 3)              # q(0,3)
        nc.sync.dma_start(out=wo_sb, in_=woT)

        v_r = v_sb.rearrange("p m (h x) -> p m h x", x=65)
        for h in range(8):
            nc.vector.memset(v_r[:, :, h, 64:65], 1.0)

        # ---- pools ----
        psp = ctx.enter_context(tc.tile_pool(name="psp", bufs=2, space="PSUM"))
        pss = ctx.enter_context(tc.tile_pool(name="pss", bufs=2, space="PSUM"))
        pso = ctx.enter_context(tc.tile_pool(name="pso", bufs=2, space="PSUM"))
        sba = ctx.enter_context(tc.tile_pool(name="sba", bufs=8))
        sbo = ctx.enter_context(tc.tile_pool(name="sbo", bufs=3))

        # ---- paced work generators (yield ~213ns PE quanta) ----
        def q_group(pr, jn):
            ps = psp.tile([128, 512], F32, tag="proj", name="qg")
            for c in range(CT):
                nc.tensor.matmul(
                    ps,
                    wq_sb[:, c, pr * 128:(pr + 1) * 128],
                    xT_sb[:, c, jn * 512:(jn + 1) * 512],
                    start=(c == 0), stop=(c == CT - 1),
                )
                yield
            nc.vector.tensor_copy(qT_sb[:, pr, jn * 512:(jn + 1) * 512], ps)

        def k_group(pr, jm):
            ps = psp.tile([128, 512], F32, tag="proj", name="kg")
            for c in range(CT):
                nc.tensor.matmul(
                    ps,
                    wk_sb[:, c, pr * 128:(pr + 1) * 128],
                    ctxT_sb[:, c, jm * 512:(jm + 1) * 512],
                    start=(c == 0), stop=(c == CT - 1),
                )
                yield
            nc.vector.tensor_copy(kT_sb[:, pr, jm * 512:(jm + 1) * 512], ps)

        def v_group(cp, i):
            # one couple = two head pairs = 4 heads (256 projection dims)
            ps = psp.tile([128, 256], F32, tag="proj", name="vg")
            for c in range(CT):
                nc.tensor.matmul(
                    ps,
                    ctxT_sb[:, c, i * 128:(i + 1) * 128],
                    wv_sb[:, c, cp * 256:(cp + 1) * 256],
                    start=(c == 0), stop=(c == CT - 1),
                )
                if c % 2 == 1:
                    yield
            nc.vector.tensor_copy(
                v_r[:, i, 4 * cp:4 * cp + 4, 0:64],
                ps.rearrange("p (h d) -> p h d", h=4),
            )

        def final_group(n128, e):
            ps = psp.tile([128, 512], F32, tag="proj", name="fg")
            for t in range(DT):
                nc.tensor.matmul(
                    ps,
                    on_sb[:, t, n128 * 128:(n128 + 1) * 128],
                    wo_sb[:, t, e * 512:(e + 1) * 512],
                    start=(t == 0), stop=(t == DT - 1),
                )
                yield
            of = sbo.tile([128, 512], F32, tag="of", name="of")
            nc.vector.tensor_copy(of, ps)
            nc.sync.dma_start(
                out=out[n128 * 128:(n128 + 1) * 128, e * 512:(e + 1) * 512],
                in_=of,
            )

        # ---- deadline-ordered pacer ----
        class Pacer:
            # At most one group is ever mid-emission (self.cur); a suspended
            # group is always finished before any other group starts, so the
            # 2-buffer proj-PSUM ring never wraps onto a live accumulation.
            def __init__(self):
                self.items = []   # list of [deadline, avail, gen]
                self.cur = None
                self._cur_dl = 10 ** 9

            def add(self, dl, avail, gen):
                self.items.append([dl, avail, gen])

            def sort(self):
                self.items.sort(key=lambda it: it[0])

            def run_due(self, s):
                due = any(it[0] <= s for it in self.items) or (
                    self.cur is not None and self._cur_dl <= s)
                if due and self.cur is not None:
                    for _ in self.cur:
                        pass
                    self.cur = None
                while True:
                    hit = None
                    for idx, it in enumerate(self.items):
                        if it[0] <= s:
                            hit = idx
                            break
                    if hit is None:
                        break
                    for _ in self.items.pop(hit)[2]:
                        pass

            def step(self, s, budget):
                for _ in range(budget):
                    while True:
                        if self.cur is None:
                            nxt = None
                            for idx, it in enumerate(self.items):
                                if it[1] <= s:
                                    nxt = self.items.pop(idx)
                                    break
                            if nxt is None:
                                return
                            self._cur_dl = nxt[0]
                            self.cur = nxt[2]
                        try:
                            next(self.cur)
                            break
                        except StopIteration:
                            self.cur = None

            def drain(self):
                if self.cur is not None:
                    for _ in self.cur:
                        pass
                    self.cur = None
                for it in self.items:
                    for _ in it[2]:
                        pass
                self.items = []

        pacer = Pacer()

        def sp(pr, j, i):
            return pr * 64 + j * 16 + i

        # pair-0 projections beyond the upfront batch (avail = conservative
        # DMA-arrival step so opportunistic runs never stall the PE queue)
        pacer.add(2, 1, k_group(0, 1))
        pacer.add(6, 5, k_group(0, 2))
        pacer.add(10, 7, k_group(0, 3))
        for i in range(MT):
            av = 3 if i < 8 else (5 if i < 12 else 7)
            pacer.add(i + LAG, av, v_group(0, i))
        pacer.add(14, 9, q_group(0, 1))
        pacer.add(30, 11, q_group(0, 2))
        pacer.add(46, 13, q_group(0, 3))
        # later pairs
        for pr in range(1, DT):
            for jn in range(NT):
                pacer.add(sp(pr, jn, 0) - 2, 14, q_group(pr, jn))
            for jm in range(4):
                pacer.add(sp(pr, 0, 4 * jm) - 2, 8, k_group(pr, jm))
        for i in range(MT):
            pacer.add(sp(2, 0, i) + LAG, 8, v_group(1, i))
        pacer.sort()
        # output projection for row block jj: available once normalize(3,jj)
        # has been emitted (step 16*jj + 211 + 1); deadline opportunistic.
        for jj in range(NT):
            av = 192 + jj * 16 + 15 + LAG + 1
            for n128 in range(jj * 4, jj * 4 + 4):
                for e in range(2):
                    pacer.add(10 ** 6, av, final_group(n128, e))

        # ---- upfront batch (hidden under the initial DMA wait) ----
        for g in [q_group(0, 0), k_group(0, 0)]:
            for _ in g:
                pass

        # ---- attention step machinery ----
        def scores(pr, j, i):
            s = pss.tile([128, 1024], F32, tag="sc", name="sc")
            for half in range(2):
                lo, hi = half * 64, half * 64 + 64
                nc.tensor.matmul(
                    s[:, half * 512:(half + 1) * 512],
                    kT_sb[lo:hi, pr, i * 128:(i + 1) * 128],
                    qT_sb[lo:hi, pr, j * 512:(j + 1) * 512],
                    start=True, stop=True,
                )
            return s

        def do_exp(s_tile):
            a = sba.tile([128, 1024], BF16, tag="attn", name="attn")
            nc.scalar.activation(a, s_tile, EXP, scale=SCALE)
            return a

        def do_av(oo, pr, i, a):
            for half in range(2):
                nc.tensor.matmul(
                    oo[half],
                    v_r[:, i, 2 * pr + half, :],
                    a[:, half * 512:(half + 1) * 512],
                    start=(i == 0), stop=(i == MT - 1),
                )

        def normalize(oo, pr, j):
            # rows 0..63 of oacc are sum(attn*v); row 64 is sum(attn).
            bcs = []
            for half in range(2):
                den = sbn.tile([1, 512], F32, tag="den", name="den")
                nc.vector.tensor_copy(den, oo[half][64:65, :])
                rec32 = sbn.tile([1, 512], F32, tag="rec32", name="rec32")
                nc.vector.reciprocal_approx_fast(out=rec32, in_=den)
                bc = sbn.tile([64, 512], F32, tag="bc", name="bc")
                nc.gpsimd.partition_broadcast(bc, rec32)
                bcs.append(bc)
            for half in range(2):
                nc.vector.tensor_mul(
                    on_sb[half * 64:half * 64 + 64, pr, j * 512:(j + 1) * 512],
                    oo[half][0:64, :], bcs[half],
                )

        # ---- linearized main loop; AV lags the exp stream by LAG steps ----
        fifo = [scores(0, 0, 0), scores(0, 0, 1)]
        afifo = []
        oo = None

        def av_step(sa):
            nonlocal oo
            pr, j, i = sa // 64, (sa // 16) % 4, sa % 16
            if i == 0:
                oo = [pso.tile([65, 512], F32, tag="oacc", name=f"oacc{h}")
                      for h in range(2)]
            do_av(oo, pr, i, afifo.pop(0))
            if i == 15:
                normalize(oo, pr, j)

        for s in range(NSTEP):
            pacer.run_due(s)
            if s + 2 < NSTEP:
                s2 = s + 2
                fifo.append(scores(s2 // 64, (s2 // 16) % 4, s2 % 16))
            afifo.append(do_exp(fifo.pop(0)))
            if s >= LAG:
                av_step(s - LAG)
            pacer.step(s, 3 if s % 4 == 3 else 2)
        for sa in range(NSTEP - LAG, NSTEP):
            av_step(sa)
        pacer.drain()


def kernel(x, context, Wq, Wk, Wv, Wo, bo):
    x = np.asarray(x, dtype=np.float32)
    context = np.asarray(context, dtype=np.float32)
    Wq = np.asarray(Wq, dtype=np.float32)
    Wk = np.asarray(Wk, dtype=np.float32)
    Wv = np.asarray(Wv, dtype=np.float32)
    Wo = np.asarray(Wo, dtype=np.float32)
    bo = np.asarray(bo, dtype=np.float32)

    if "nc" not in _CACHE:
        _CACHE["nc"] = _build_program()
    nc = _CACHE["nc"]

    in_maps = _make_in_maps(x, context, Wq, Wk, Wv, Wo)
    res = bass_utils.run_bass_kernel_spmd(nc, in_maps, core_ids=list(range(NCORES)))

    final = np.empty((B, N, DIM), dtype=np.float32)
    for b in range(B):
        final[b] = res.results[2 * b]["out"] + res.results[2 * b + 1]["out"] + bo
    return final


def _img_w(a):
    # DRAM->SBUF weight image: (K=1024, F) -> (128, K//128, F), p-major
    return np.ascontiguousarray(
        a.reshape(-1, 128, a.shape[1]).transpose(1, 0, 2))


def _img_x(a):
    # activation image, column-chunked: (1024, 2048) -> (4, 128, 8, 512)
    return np.ascontiguousarray(
        a.reshape(CT, 128, 4, 512).transpose(2, 1, 0, 3))


def _make_in_maps(x, context, Wq, Wk, Wv, Wo):
    bf = ml_dtypes.bfloat16
    xT = [_img_x(np.ascontiguousarray(x[b].T).astype(bf)) for b in range(B)]
    ctxT = [_img_x(np.ascontiguousarray(context[b].T).astype(bf)) for b in range(B)]
    wT = {}
    for g in range(2):
        sl = slice(g * HG, (g + 1) * HG)
        wT[g] = {
            "wqT": _img_w(np.ascontiguousarray(Wq[sl, :].T).astype(bf)),
            "wkT": _img_w(np.ascontiguousarray(Wk[sl, :].T).astype(bf)),
            "wvT": _img_w(np.ascontiguousarray(Wv[sl, :].T).astype(bf)),
            "woT": _img_w(np.ascontiguousarray(Wo[:, sl].T).astype(bf)),
        }
    in_maps = []
    for c in range(NCORES):
        b, g = c // 2, c % 2
        m = {"xT": xT[b], "ctxT": ctxT[b]}
        m.update(wT[g])
        in_maps.append(m)
    return in_maps


def timed_run(inp, trace_dir=None):
    """Run with NTFF tracing; returns HW exec time in ns (or None)."""
    if "nc" not in _CACHE:
        _CACHE["nc"] = _build_program()
    nc = _CACHE["nc"]
    in_maps = _make_in_maps(
        np.asarray(inp["x"], np.float32), np.asarray(inp["context"], np.float32),
        np.asarray(inp["Wq"], np.float32), np.asarray(inp["Wk"], np.float32),
        np.asarray(inp["Wv"], np.float32), np.asarray(inp["Wo"], np.float32))
    res = bass_utils.run_bass_kernel_spmd(
        nc, in_maps, core_ids=list(range(NCORES)), trace=True, tmpdir=trace_dir)
    return res.exec_time_ns
